# revision 1
# baseline (speedup 1.0000x reference)
"""Trainium2 Bass kernel for an attention-GRU cell (Bahdanau attention + GRU update).

Computation (per batch row b):
    x   = inputs @ Wi + bi
    xg  = x @ kernel + bias                       (split into x_z, x_r, x_h)
    q   = h_tm1 @ Ua + ba_u
    S   = tanh(context @ Wa + ba_w + q)           [t, U]
    sc  = S @ Va + ba_v                           [t]
    attn = softmax(sc)                            (scores bounded by ||Va||_1 -> no max-sub)
    cv  = sum_t attn * context                    [U]
    cg  = cv @ attention_kernel                   (c_z, c_r, c_h)
    z   = sigmoid(x_z + h@Rz + c_z) ; r = sigmoid(x_r + h@Rr + c_r)
    hb  = tanh(x_h + (r*h)@Rh + c_h)
    h   = z*h_tm1 + (1-z)*hb ; out = h @ Wo + bo

Sharding: batch (64) split across 8 cores, 8 batches/core, weights replicated.
Each core is fully independent (no collectives). Context path runs in fp16
(measured end-to-end error ~3e-4 of out absmax vs the fp32 reference).

Layout strategy per core (measured ~320us on HW; HBM roofline ~100us):
  - context batch slice DMA-loaded with fp32->fp16 cast (SWDGE), laid out
    partition-contiguous: nat[p, j, u] = ctx[16p+j, u] (fast descriptors).
    All t-indexing downstream inherits this scrambled order consistently
    (softmax is permutation-invariant; ctxT rows and replicated-attn rows
    pair up in the same order), so correctness is unaffected.
  - ctxT produced by PE transposes (identity matmul, ~60-90ns per 128x128
    f16 tile) packed into f16 PSUM tiles, unpacked to SBUF on the Scalar
    engine (keeps the Vector engine off the transpose critical path).
    Transposes are interleaved with score matmuls to keep the PE HAM warm.
  - scores matmul: Wa stationary, ctxT moving -> S^T chunks in PSUM;
    tanh on ACT with per-partition bias = (q + ba_w)^T
  - Va dot via matmul (lhsT = Va chunk [128,1]) -> scores [1, t] in PSUM
  - exp on ACT with fused accum_out -> softmax normalizer (no max-subtract
    needed: |score| <= ||Va||_1 ~ 8, exp stays in fp32 range)
  - attn replicated across partitions (gpsimd partition_broadcast); ctx_vec
    computed on the Vector engine via scalar_tensor_tensor with accum_out
    (fused multiply+reduce over ctxT; NOTE tensor_tensor_reduce hangs the
    device in this environment, scalar_tensor_tensor works)
  - gate math per 4-batch group on partitions 0..3 (SBUF compute APs may
    only start at partitions {0,32,64,96}); group-post emission is delayed
    behind the next batch's work to avoid PE head-of-line blocking
"""

import sys

if "/opt/trn_rl_repo" not in sys.path:
    sys.path.insert(0, "/opt/trn_rl_repo")

import numpy as np

import concourse.bass as bass
import concourse.mybir as mybir
import concourse.tile as tile
from concourse import bacc

F32 = mybir.dt.float32
F16 = mybir.dt.float16
AF = mybir.ActivationFunctionType
OP = mybir.AluOpType

B = 64          # total batch
T = 2048        # context length
U = 512         # units
EMB = 256
NCORES = 8
BPC = B // NCORES   # batches per core
KU = U // 128       # 4 k-chunks over units
TC = T // 128       # 16 t-chunks


def _build_program():
    nc = bacc.Bacc("TRN2", target_bir_lowering=False, debug=False, num_devices=NCORES)

    # ---- DRAM I/O ----
    ctx_d = nc.dram_tensor("ctx", [BPC, T, U], F32, kind="ExternalInput").ap()
    inp_d = nc.dram_tensor("inp", [BPC, EMB], F32, kind="ExternalInput").ap()
    h0_d = nc.dram_tensor("h0", [BPC, U], F32, kind="ExternalInput").ap()

    wa_d = nc.dram_tensor("wa16", [U, U], F16, kind="ExternalInput").ap()
    wa8_d = nc.dram_tensor("wa8dr", [128, 2, 2, KU, 128], mybir.dt.float8e4,
                           kind="ExternalInput").ap()
    ua_d = nc.dram_tensor("ua16", [U, U], F16, kind="ExternalInput").ap()
    wi_d = nc.dram_tensor("wi16", [EMB, U], F16, kind="ExternalInput").ap()
    kern_d = nc.dram_tensor("kern16", [U, 3 * U], F16, kind="ExternalInput").ap()
    rec_d = nc.dram_tensor("rec16", [U, 3 * U], F16, kind="ExternalInput").ap()
    attk_d = nc.dram_tensor("attk16", [U, 3 * U], F16, kind="ExternalInput").ap()
    wo_d = nc.dram_tensor("wo16", [U, U], F16, kind="ExternalInput").ap()
    vat_d = nc.dram_tensor("va_t", [128, KU], F16, kind="ExternalInput").ap()
    id_d = nc.dram_tensor("ident16", [128, 128], F16, kind="ExternalInput").ap()

    bi_d = nc.dram_tensor("bi", [U], F32, kind="ExternalInput").ap()
    bg_d = nc.dram_tensor("biasg", [3 * U], F32, kind="ExternalInput").ap()
    bau_d = nc.dram_tensor("ba_u", [U], F32, kind="ExternalInput").ap()
    bawt_d = nc.dram_tensor("ba_wt8", [128, KU, BPC], F32, kind="ExternalInput").ap()
    bav_d = nc.dram_tensor("ba_v1", [1, 1], F32, kind="ExternalInput").ap()
    bo_d = nc.dram_tensor("bo", [U], F32, kind="ExternalInput").ap()

    out_d = nc.dram_tensor("out_o", [BPC, U], F32, kind="ExternalOutput").ap()
    h_d = nc.dram_tensor("h_o", [BPC, U], F32, kind="ExternalOutput").ap()

    with tile.TileContext(nc) as tc:
        _emit(nc, tc, locals())
    nc.compile()
    return nc


def _bcast_rows(ap_1d, rows, cols):
    """DMA source AP replicating a 1-D [cols] dram tensor across `rows` partitions."""
    return bass.AP(ap_1d.tensor, 0, [[0, rows], [1, cols]])


def _emit(nc, tc, d):
    ctx_d, inp_d, h0_d = d["ctx_d"], d["inp_d"], d["h0_d"]
    wa_d, ua_d, wi_d, kern_d = d["wa_d"], d["ua_d"], d["wi_d"], d["kern_d"]
    wa8_d = d["wa8_d"]
    rec_d, attk_d, wo_d, vat_d, id_d = (
        d["rec_d"], d["attk_d"], d["wo_d"], d["vat_d"], d["id_d"],
    )
    bi_d, bg_d, bau_d, bawt_d, bav_d, bo_d = (
        d["bi_d"], d["bg_d"], d["bau_d"], d["bawt_d"], d["bav_d"], d["bo_d"],
    )
    out_d, h_d = d["out_d"], d["h_d"]

    from contextlib import ExitStack

    es = ExitStack()
    wp = es.enter_context(tc.tile_pool(name="weights", bufs=1))
    gp = es.enter_context(tc.tile_pool(name="group", bufs=2))
    bp = es.enter_context(tc.tile_pool(name="perbatch", bufs=2))
    natTp = es.enter_context(tc.tile_pool(name="natT", bufs=2))
    thp = es.enter_context(tc.tile_pool(name="tanh", bufs=4))
    # psT shares pool pp (1 bank) with small matmul tiles
    # PSUM budget: 8 banks. psS [128,1024]f32=2bk x2; psSC [1,1024]=2bk? x1; psSM 1bk x2
    pS = es.enter_context(tc.tile_pool(name="psS", bufs=2, space="PSUM"))
    pSC = es.enter_context(tc.tile_pool(name="psSC", bufs=1, space="PSUM"))
    pT = es.enter_context(tc.tile_pool(name="psT", bufs=2, space="PSUM"))
    pp = pT  # unified small-psum pool; all tiles share one 1-bank slot tag

    # ---- one-time loads (weights used in steady state) ----
    def load_kxm(pool, dram, rows, cols, tag):
        t = pool.tile([128, rows // 128, cols], F16, tag=tag, name=tag)
        src = bass.AP(dram.tensor, 0, [[cols, 128], [128 * cols, rows // 128], [1, cols]])
        nc.sync.dma_start(out=t, in_=src)
        return t

    id_sb = wp.tile([128, 128], F16)
    nc.sync.dma_start(out=id_sb, in_=id_d)
    va_sb = wp.tile([128, KU], F16)
    nc.sync.dma_start(out=va_sb, in_=vat_d)
    wa8_sb = wp.tile([128, 2, 2, KU, 128], mybir.dt.float8e4)
    nc.sync.dma_start(out=wa8_sb, in_=wa8_d)

    # prefetch the first two context batches before the bulk weight loads
    natp = es.enter_context(tc.tile_pool(name="nat", bufs=2))
    nat_pre = {}
    for pb_ in range(2):
        t = natp.tile([128, TC, U], F16, tag="nat", name=f"natp{pb_}")
        nc.gpsimd.dma_start(out=t, in_=bass.AP(
            ctx_d.tensor, pb_ * T * U, [[TC * U, 128], [U, TC], [1, U]]))
        nat_pre[pb_] = t

    rec_sb = load_kxm(wp, rec_d, U, 3 * U, "recw")
    attk_sb = load_kxm(wp, attk_d, U, 3 * U, "attkw")
    wo_sb = load_kxm(wp, wo_d, U, U, "wow")

    bo4 = wp.tile([4, U], F32)
    nc.sync.dma_start(out=bo4, in_=_bcast_rows(bo_d, 4, U))
    bawt8 = wp.tile([128, KU, BPC], F32)
    nc.sync.dma_start(out=bawt8, in_=bawt_d)
    bav_sb = wp.tile([1, 1], F32)
    nc.sync.dma_start(out=bav_sb, in_=bav_d)

    # h_tm1 per group halves (partition slices >=4 are illegal on SBUF APs)
    h032g = []
    for g in range(2):
        t = wp.tile([4, U], F32, tag=f"h032g{g}", name=f"h032g{g}")
        nc.sync.dma_start(out=t, in_=h0_d[g * 4:(g + 1) * 4, :])
        h032g.append(t)

    # ---- helpers ----
    def transpose_to(dst_f16, src, nrow, chunks):
        """PE-transpose src [nrow, chunks*128] f16 -> dst [128, chunks*nrow] f16."""
        pm = pp.tile([128, chunks * nrow], F16, tag="u", name="pm")
        for c in range(chunks):
            nc.tensor.transpose(
                pm[:, c * nrow:(c + 1) * nrow],
                src[0:nrow, c * 128:(c + 1) * 128],
                id_sb[0:nrow, 0:nrow],
            )
        nc.vector.tensor_copy(dst_f16, pm[:, 0:chunks * nrow])

    # ---- resident per-core intermediates ----
    qb = wp.tile([128, KU, BPC], F32)           # tanh bias (q + ba_w)^T
    xgg = [wp.tile([4, 3 * U], F32, tag=f"xg{g}", name=f"xg{g}") for g in range(2)]
    xgrzg = [wp.tile([4, 2 * U], F32, tag=f"xz{g}", name=f"xz{g}") for g in range(2)]

    # ---- phase 0 (scoped SBUF, reclaimed afterwards) ----
    with tc.tile_pool(name="phase0", bufs=1) as p0:
        ua_sb = load_kxm(p0, ua_d, U, U, "uaw")
        wi_sb = load_kxm(p0, wi_d, EMB, U, "wiw")
        kern_sb = load_kxm(p0, kern_d, U, 3 * U, "kernw")

        bi8 = p0.tile([BPC, U], F32)
        nc.sync.dma_start(out=bi8, in_=_bcast_rows(bi_d, BPC, U))
        bg4 = p0.tile([4, 3 * U], F32)
        nc.sync.dma_start(out=bg4, in_=_bcast_rows(bg_d, 4, 3 * U))
        bau8 = p0.tile([BPC, U], F32)
        nc.sync.dma_start(out=bau8, in_=_bcast_rows(bau_d, BPC, U))
        inp16 = p0.tile([BPC, EMB], F16)
        nc.gpsimd.dma_start(out=inp16, in_=inp_d)
        h016 = p0.tile([BPC, U], F16)
        nc.gpsimd.dma_start(out=h016, in_=h0_d)

        inT = p0.tile([128, 2 * BPC], F16)      # layout [c, row]
        transpose_to(inT, inp16, BPC, 2)
        hT = p0.tile([128, KU * BPC], F16)
        transpose_to(hT, h016, BPC, KU)

        # x = inputs @ Wi + bi
        px = pp.tile([BPC, U], F32, tag="u", name="px")
        for c in range(2):
            nc.tensor.matmul(px, inT[:, c * BPC:(c + 1) * BPC], wi_sb[:, c, :],
                             start=(c == 0), stop=(c == 1))
        x16 = p0.tile([BPC, U], F16)
        nc.vector.tensor_add(x16, px, bi8)
        xT = p0.tile([128, KU * BPC], F16)
        transpose_to(xT, x16, BPC, KU)

        # xg / rec_zr computed per 4-batch half (legal partition starts)
        for g in range(2):
            for n in range(3):
                pg = pp.tile([4, U], F32, tag="u", name="pg")
                for c in range(KU):
                    nc.tensor.matmul(pg, xT[:, c * BPC + 4 * g:c * BPC + 4 * g + 4],
                                     kern_sb[:, c, n * U:(n + 1) * U],
                                     start=(c == 0), stop=(c == KU - 1))
                nc.vector.tensor_add(xgg[g][:, n * U:(n + 1) * U], pg,
                                     bg4[:, n * U:(n + 1) * U])
            for n in range(2):
                pr = pp.tile([4, U], F32, tag="u", name="pr")
                for c in range(KU):
                    nc.tensor.matmul(pr, hT[:, c * BPC + 4 * g:c * BPC + 4 * g + 4],
                                     rec_sb[:, c, n * U:(n + 1) * U],
                                     start=(c == 0), stop=(c == KU - 1))
                nc.vector.tensor_add(xgrzg[g][:, n * U:(n + 1) * U], pr,
                                     xgg[g][:, n * U:(n + 1) * U])

        # q = h @ Ua + ba_u ; transposed, +ba_w -> tanh bias [128, KU, BPC]
        pq = pp.tile([BPC, U], F32, tag="u", name="pq")
        for c in range(KU):
            nc.tensor.matmul(pq, hT[:, c * BPC:(c + 1) * BPC], ua_sb[:, c, :],
                             start=(c == 0), stop=(c == KU - 1))
        q16 = p0.tile([BPC, U], F16)
        nc.vector.tensor_add(q16, pq, bau8)
        pmq = pp.tile([128, KU * BPC], F16, tag="u", name="pmq")
        for c in range(KU):
            nc.tensor.transpose(pmq[:, c * BPC:(c + 1) * BPC],
                                q16[0:BPC, c * 128:(c + 1) * 128],
                                id_sb[0:BPC, 0:BPC])
        for c in range(KU):
            nc.vector.tensor_add(qb[:, c, :], pmq[:, c * BPC:(c + 1) * BPC],
                                 bawt8[:, c, :])

    def emit_group_post(grp, cvT16, h032, xg):

        # ---- group post: cg, gates, h, out ----

        def mm_group(lhsT4, rhs_w, ncol_off):
            ptile = pp.tile([4, U], F32, tag="u", name="ptile")
            for c in range(KU):
                nc.tensor.matmul(ptile, lhsT4[:, c, :],
                                 rhs_w[:, c, ncol_off:ncol_off + U],
                                 start=(c == 0), stop=(c == KU - 1))
            return ptile

        def sigmoid4(dst, pre):
            t1 = gp.tile([4, U], F32, tag="sig_t")
            nc.scalar.activation(t1, pre, AF.Tanh, scale=0.5)
            nc.vector.tensor_scalar(dst, t1, 0.5, 0.5, OP.mult, OP.add)

        xgrz = xgrzg[grp]
        # z gate
        pcg_z = mm_group(cvT16, attk_sb, 0)
        zpre = gp.tile([4, U], F32, tag="zpre")
        nc.vector.scalar_tensor_tensor(zpre, pcg_z, 1.0, xgrz[:, 0:U],
                                       OP.mult, OP.add)
        zg = gp.tile([4, U], F32, tag="zg")
        sigmoid4(zg, zpre)

        # r gate
        pcg_r = mm_group(cvT16, attk_sb, U)
        rpre = gp.tile([4, U], F32, tag="rpre")
        nc.vector.scalar_tensor_tensor(rpre, pcg_r, 1.0, xgrz[:, U:2 * U],
                                       OP.mult, OP.add)
        rg = gp.tile([4, U], F32, tag="rg")
        sigmoid4(rg, rpre)

        # rec_h = (r*h) @ Rh
        rh16 = gp.tile([4, U], F16, tag="rh16")
        nc.vector.tensor_mul(rh16, rg, h032)
        rhT = gp.tile([128, KU, 4], F16, tag="rhT")
        pmr = pp.tile([128, KU * 4], F16, tag="u", name="pmr")
        for c in range(KU):
            nc.tensor.transpose(pmr[:, c * 4:(c + 1) * 4],
                                rh16[0:4, c * 128:(c + 1) * 128], id_sb[0:4, 0:4])
        nc.vector.tensor_copy(rhT, pmr[:, 0:KU * 4])
        prh = mm_group(rhT, rec_sb, 2 * U)

        # h_bar
        hpre = gp.tile([4, U], F32, tag="hpre")
        nc.vector.scalar_tensor_tensor(hpre, prh, 1.0, xg[:, 2 * U:3 * U],
                                       OP.mult, OP.add)
        pcg_h = mm_group(cvT16, attk_sb, 2 * U)
        nc.vector.tensor_add(hpre, hpre, pcg_h)
        hbar = gp.tile([4, U], F32, tag="hbar")
        nc.scalar.activation(hbar, hpre, AF.Tanh)

        # h = hbar + z*(h_tm1 - hbar)
        dd = gp.tile([4, U], F32, tag="dd")
        nc.vector.tensor_sub(dd, h032, hbar)
        h_out = gp.tile([4, U], F32, tag="h_out")
        nc.vector.scalar_tensor_tensor(h_out, dd, 1.0, zg, OP.mult, OP.mult)
        nc.vector.tensor_add(h_out, h_out, hbar)
        nc.sync.dma_start(out=h_d[grp * 4:(grp + 1) * 4, :], in_=h_out)

        # out = h @ Wo + bo
        h16 = gp.tile([4, U], F16, tag="h16")
        nc.vector.tensor_copy(h16, h_out)
        hT4 = gp.tile([128, KU, 4], F16, tag="hT4")
        pmh = pp.tile([128, KU * 4], F16, tag="u", name="pmh")
        for c in range(KU):
            nc.tensor.transpose(pmh[:, c * 4:(c + 1) * 4],
                                h16[0:4, c * 128:(c + 1) * 128], id_sb[0:4, 0:4])
        nc.vector.tensor_copy(hT4, pmh[:, 0:KU * 4])
        pout = mm_group(hT4, wo_sb, 0)
        o_out = gp.tile([4, U], F32, tag="o_out")
        nc.vector.tensor_add(o_out, pout, bo4)
        nc.sync.dma_start(out=out_d[grp * 4:(grp + 1) * 4, :], in_=o_out)


    # ---- streaming over batches ----
    # nat[p, j, u] = ctx[b, 16p+j, u]: per-partition contiguous load (fast
    # SWDGE descriptors). All downstream t-indexing inherits this scrambled
    # order consistently (softmax is permutation-invariant; ctxT and the
    # replicated attn rows pair up in the same order).

    cvT32 = None
    cvT16 = None
    zrecRep = None
    pending = []
    for b in range(BPC):
        gi = b % 4
        grp = b // 4
        if gi == 0:
            cvT32 = gp.tile([128, KU, 4], F32, tag="cvT32")
            cvT16 = gp.tile([128, KU, 4], F16, tag="cvT16")
            zrecRep = gp.tile([128, 4], F32, tag="zrecRep")

        if b in nat_pre:
            nat = nat_pre.pop(b)
        else:
            nat = natp.tile([128, TC, U], F16, tag="nat")
            nc.gpsimd.dma_start(out=nat, in_=bass.AP(
                ctx_d.tensor, b * T * U, [[TC * U, 128], [U, TC], [1, U]]))

        zp = bp.tile([1, 2], F32, tag="zpb")
        cvPart = bp.tile([128, KU], F32, tag="cvPart")
        cvPartB = bp.tile([128, KU], F32, tag="cvPartB")

        for th in range(2):
            natT = natTp.tile([128, KU, 1024], mybir.dt.float8e4, tag="natT")

            def tp_pair(pair):
                psT = pT.tile([128, 1024], F16, tag="u", name="psT")
                for jj in range(2):
                    j = th * 8 + pair * 2 + jj
                    for uc in range(KU):
                        nc.tensor.transpose(
                            psT[:, jj * 512 + uc * 128: jj * 512 + (uc + 1) * 128],
                            nat[:, j, uc * 128:(uc + 1) * 128],
                            id_sb,
                        )
                for jj in range(2):
                    lj = pair * 2 + jj
                    nc.scalar.copy(
                        natT.rearrange("a uc (jx p) -> a uc jx p", p=128)[:, :, lj, :],
                        psT.rearrange("a (jx uc p) -> a jx uc p", uc=KU, p=128)[:, jj, :, :],
                    )

            def score_mms(ps_tiles, ms, half):
                for mi, m in enumerate(ms):
                    for c in range(2):
                        nc.tensor.matmul(
                            ps_tiles[mi][:, half * 512:(half + 1) * 512],
                            wa8_sb[:, c, :, m, :],
                            natT[:, 2 * c:2 * c + 2, half * 512:(half + 1) * 512],
                            start=(c == 0), stop=(c == 1),
                            perf_mode=mybir.MatmulPerfMode.DoubleRow,
                        )

            psc = pSC.tile([1, 1024], F32, tag="sc")
            th16s = [None] * KU
            # interleave: transposes for half0, scores m01-half0, transposes
            # half1, scores m01-half1, tanh, then m23 both halves, tanh, Va
            tp_pair(0); tp_pair(1)
            ps01 = [pS.tile([128, 1024], F32, tag="S", name=f"ps{mm}") for mm in range(2)]
            score_mms(ps01, [0, 1], 0)
            tp_pair(2); tp_pair(3)
            score_mms(ps01, [0, 1], 1)
            for mi, m in enumerate([0, 1]):
                th16 = thp.tile([128, 1024], F16, tag="th", name=f"th16_{m}")
                nc.scalar.activation(th16, ps01[mi], AF.Tanh, scale=1.0 / 16.0,
                                     bias=qb[:, m, b:b + 1])
                th16s[m] = th16
            ps23 = [pS.tile([128, 1024], F32, tag="S", name=f"ps{mm + 2}") for mm in range(2)]
            score_mms(ps23, [2, 3], 0)
            score_mms(ps23, [2, 3], 1)
            for mi, m in enumerate([2, 3]):
                th16 = thp.tile([128, 1024], F16, tag="th", name=f"th16_{m}")
                nc.scalar.activation(th16, ps23[mi], AF.Tanh, scale=1.0 / 16.0,
                                     bias=qb[:, m, b:b + 1])
                th16s[m] = th16
            for half in range(2):
                for m in range(KU):
                    nc.tensor.matmul(
                        psc[0:1, half * 512:(half + 1) * 512],
                        va_sb[:, m:m + 1],
                        th16s[m][:, half * 512:(half + 1) * 512],
                        start=(m == 0), stop=(m == KU - 1),
                    )
            expTh = bp.tile([1, 1024], F16, tag="expTh")
            nc.scalar.activation(expTh, psc, AF.Exp, bias=bav_sb[0:1, 0:1],
                                 accum_out=zp[0:1, th:th + 1])

            # replicate exp row across partitions
            expRep = bp.tile([128, 1024], F16, tag="expRep")
            nc.gpsimd.partition_broadcast(expRep, expTh[0:1, :])

            # cv partial: cvU[u] += sum_t ctxT[u, t] * exp[t]
            dump = bp.tile([128, 1024], F16, tag="dump")
            cvdst = cvPart if th == 0 else cvPartB
            for uc in range(KU):
                nc.vector.scalar_tensor_tensor(
                    dump, natT[:, uc, :], 1.0, expRep,
                    OP.mult, OP.mult, accum_out=cvdst[:, uc:uc + 1])

        # 1/Z, replicated to a column of zrecRep
        zrec = bp.tile([1, 1], F32, tag="zrec")
        nc.vector.tensor_add(zrec, zp[:, 0:1], zp[:, 1:2])
        nc.vector.reciprocal(zrec, zrec)
        nc.gpsimd.partition_broadcast(zrecRep[:, gi:gi + 1], zrec[0:1, :])
        nc.vector.tensor_add(cvT32[:, :, gi:gi + 1], cvPart, cvPartB)
        nc.vector.tensor_scalar(cvT16[:, :, gi:gi + 1], cvT32[:, :, gi:gi + 1],
                                zrecRep[:, gi:gi + 1], None, OP.mult)

        if gi == 3:
            pending.append((grp, cvT16))
        if len(pending) and b % 4 == 1 and b > 1:
            g0, cv0 = pending.pop(0)
            emit_group_post(g0, cv0, h032g[g0], xgg[g0])

    while pending:
        g0, cv0 = pending.pop(0)
        emit_group_post(g0, cv0, h032g[g0], xgg[g0])

    es.close()


_PROGRAM = None


def _get_program():
    global _PROGRAM
    if _PROGRAM is None:
        _PROGRAM = _build_program()
    return _PROGRAM


def make_in_maps(inputs, h_tm1, context, Wi, bi, kernel, recurrent_kernel,
                 attention_kernel, bias, Wa, ba_w, Ua, ba_u, Va, ba_v, Wo, bo):
    f32 = lambda x: np.ascontiguousarray(np.asarray(x, dtype=np.float32))
    f16 = lambda x: np.ascontiguousarray(np.asarray(x, dtype=np.float32).astype(np.float16))

    context = f32(context)
    inputs = f32(inputs)
    h_tm1 = f32(h_tm1)

    wa32 = np.asarray(Wa, np.float32) * 16.0
    f8 = mybir.dt.np(mybir.dt.float8e4)
    wa8dr = np.zeros((128, 2, 2, KU, 128), np.float32)
    for c in range(2):
        for i in range(2):
            for mc in range(KU):
                # lhsT[p, i, m] = Wa'[c*256 + i*128 + p, mc*128 + m]
                wa8dr[:, c, i, mc, :] = wa32[c * 256 + i * 128: c * 256 + (i + 1) * 128,
                                             mc * 128:(mc + 1) * 128]
    shared = {
        "wa8dr": np.ascontiguousarray(wa8dr.astype(f8)),
        "wa16": f16(Wa), "ua16": f16(Ua), "wi16": f16(Wi),
        "kern16": f16(kernel), "rec16": f16(recurrent_kernel),
        "attk16": f16(attention_kernel), "wo16": f16(Wo),
        "va_t": np.ascontiguousarray(
            np.asarray(Va, np.float32).reshape(KU, 128).T.astype(np.float16)),
        "ident16": np.eye(128, dtype=np.float16),
        "bi": f32(bi), "biasg": f32(bias), "ba_u": f32(ba_u),
        "ba_wt8": np.ascontiguousarray(np.repeat(
            np.asarray(ba_w, np.float32).reshape(KU, 128).T[:, :, None], BPC, axis=2)),
        "ba_v1": f32(ba_v).reshape(1, 1),
        "bo": f32(bo),
    }
    in_maps = []
    for i in range(NCORES):
        s = slice(i * BPC, (i + 1) * BPC)
        in_maps.append({
            "ctx": context[s], "inp": inputs[s], "h0": h_tm1[s], **shared,
        })
    return in_maps


def kernel(**inputs):
    from concourse.bass_utils import run_bass_kernel_spmd

    nc = _get_program()
    in_maps = make_in_maps(**inputs)
    res = run_bass_kernel_spmd(nc, in_maps, list(range(NCORES)))
    out = np.concatenate([r["out_o"] for r in res.results], axis=0)
    h = np.concatenate([r["h_o"] for r in res.results], axis=0)
    return out.astype(np.float32), h.astype(np.float32)


if __name__ == "__main__":
    prog = _get_program()
    print("program built OK:", len(prog.m.functions[0].instructions) if hasattr(prog.m.functions[0], "instructions") else "?")



# revision 10
# speedup vs baseline: 1.4301x; 1.4301x over previous
"""Trainium2 Bass kernel for an attention-GRU cell (Bahdanau attention + GRU update).

Computation (per batch row b):
    x   = inputs @ Wi + bi
    xg  = x @ kernel + bias                       (split into x_z, x_r, x_h)
    q   = h_tm1 @ Ua + ba_u
    S   = tanh(context @ Wa + ba_w + q)           [t, U]
    sc  = S @ Va + ba_v                           [t]
    attn = softmax(sc)                            (scores bounded -> no max-sub)
    cv  = sum_t attn * context                    [U]
    cg  = cv @ attention_kernel                   (c_z, c_r, c_h)
    z   = sigmoid(x_z + h@Rz + c_z) ; r = sigmoid(x_r + h@Rr + c_r)
    hb  = tanh(x_h + (r*h)@Rh + c_h)
    h   = z*h_tm1 + (1-z)*hb ; out = h @ Wo + bo

Sharding: batch (64) split across 8 cores, 8 batches/core, weights replicated.
Each core fully independent (no collectives).

v2 design (v1 measured 328us; steady state was ACT-bound at ~13.1us per
t-half: 8 PSUM->SBUF unpack copies + 4 tanh + exp all on the Scalar engine,
plus a 45us serialized weight-load startup):
  - context loaded as f8e4 directly (SWDGE f32->f8 cast).  v1 already fed f8
    into both the score matmul and the cv reduction, so this loses nothing.
  - PE transposes run in f8; unpack copies (PSUM->SBUF) moved to the Vector
    engine (2 copies of [128,2048] per t-half), freeing ACT for tanh/exp only.
  - Va-dot uses DoubleRow f8 with a partition-replicated Va stationary, so
    the score row lands replicated across all 128 partitions in PSUM; exp
    reads it directly.  This kills both gpsimd partition_broadcasts of v1.
  - tanh outputs f8 (feeds the f8 DoubleRow Va matmul).
  - cv = sum_t exp[t]*ctxT[u,t] runs on GpSimd (scalar_tensor_tensor with
    accum_out), freeing the Vector engine for the unpack copies.
  - gate/post math is M=8 (all 8 batches at once, partition 0-7) instead of
    2 groups of 4: halves the PE instruction count there and avoids illegal
    partition-4 AP starts.
  - weight loads spread across sync + scalar HWDGE queues, big tensors last,
    so phase0's q-chain starts ~5us in instead of ~45us.
"""

import sys

if "/opt/trn_rl_repo" not in sys.path:
    sys.path.insert(0, "/opt/trn_rl_repo")

import numpy as np

import concourse.bass as bass
import concourse.mybir as mybir
import concourse.tile as tile
from concourse import bacc

F32 = mybir.dt.float32
F16 = mybir.dt.float16
F8 = mybir.dt.float8e4
AF = mybir.ActivationFunctionType
OP = mybir.AluOpType
DR = mybir.MatmulPerfMode.DoubleRow

B = 64          # total batch
T = 2048        # context length
U = 512         # units
EMB = 256
NCORES = 8
BPC = B // NCORES   # batches per core
KU = U // 128       # 4 k-chunks over units
TC = T // 128       # 16 t-chunks
TH = 1024           # t positions per half


def _build_program():
    nc = bacc.Bacc("TRN2", target_bir_lowering=False, debug=False, num_devices=NCORES)

    # ---- DRAM I/O ----
    ctx_d = nc.dram_tensor("ctx", [BPC, T, U], F32, kind="ExternalInput").ap()
    inp_d = nc.dram_tensor("inp", [BPC, EMB], F32, kind="ExternalInput").ap()
    h0_d = nc.dram_tensor("h0", [BPC, U], F32, kind="ExternalInput").ap()

    wa8_d = nc.dram_tensor("wa8dr", [128, 2, 2, KU, 128], F8, kind="ExternalInput").ap()
    va8_d = nc.dram_tensor("va8rep", [128, KU, 128], F8, kind="ExternalInput").ap()
    id16_d = nc.dram_tensor("ident16", [128, 128], F16, kind="ExternalInput").ap()

    ua_d = nc.dram_tensor("ua16", [U, U], F16, kind="ExternalInput").ap()
    wi_d = nc.dram_tensor("wi16", [EMB, U], F16, kind="ExternalInput").ap()
    kern_d = nc.dram_tensor("kern16", [U, 3 * U], F16, kind="ExternalInput").ap()
    rec_d = nc.dram_tensor("rec16", [U, 3 * U], F16, kind="ExternalInput").ap()
    attk_d = nc.dram_tensor("attk16", [U, 3 * U], F16, kind="ExternalInput").ap()
    wo_d = nc.dram_tensor("wo16", [U, U], F16, kind="ExternalInput").ap()

    bi_d = nc.dram_tensor("bi", [U], F32, kind="ExternalInput").ap()
    bg_d = nc.dram_tensor("biasg", [3 * U], F32, kind="ExternalInput").ap()
    bau_d = nc.dram_tensor("ba_u", [U], F32, kind="ExternalInput").ap()
    bawt_d = nc.dram_tensor("ba_wt8", [128, KU, BPC], F32, kind="ExternalInput").ap()
    bav_d = nc.dram_tensor("ba_v1", [1, 1], F32, kind="ExternalInput").ap()
    bo_d = nc.dram_tensor("bo", [U], F32, kind="ExternalInput").ap()

    out_d = nc.dram_tensor("out_o", [BPC, U], F32, kind="ExternalOutput").ap()
    h_d = nc.dram_tensor("h_o", [BPC, U], F32, kind="ExternalOutput").ap()

    with tile.TileContext(nc) as tc:
        _emit(nc, tc, locals())
    nc.compile()
    return nc


def _bcast_rows(ap_1d, rows, cols):
    """DMA source AP replicating a 1-D [cols] dram tensor across `rows` partitions."""
    return bass.AP(ap_1d.tensor, 0, [[0, rows], [1, cols]])


def _emit(nc, tc, d):
    ctx_d, inp_d, h0_d = d["ctx_d"], d["inp_d"], d["h0_d"]
    wa8_d, va8_d, id16_d = d["wa8_d"], d["va8_d"], d["id16_d"]
    ua_d, wi_d, kern_d, rec_d, attk_d, wo_d = (
        d["ua_d"], d["wi_d"], d["kern_d"], d["rec_d"], d["attk_d"], d["wo_d"],
    )
    bi_d, bg_d, bau_d, bawt_d, bav_d, bo_d = (
        d["bi_d"], d["bg_d"], d["bau_d"], d["bawt_d"], d["bav_d"], d["bo_d"],
    )
    out_d, h_d = d["out_d"], d["h_d"]

    from contextlib import ExitStack

    es = ExitStack()
    wp = es.enter_context(tc.tile_pool(name="weights", bufs=1))
    gp = es.enter_context(tc.tile_pool(name="group", bufs=2))
    bp = es.enter_context(tc.tile_pool(name="perbatch", bufs=2))
    natp = es.enter_context(tc.tile_pool(name="nat", bufs=2))
    ntp = es.enter_context(tc.tile_pool(name="natT", bufs=2))
    thp = es.enter_context(tc.tile_pool(name="th8", bufs=2))
    erp = es.enter_context(tc.tile_pool(name="exprep", bufs=2))
    # PSUM budget 8 banks: pS 2x[128,1024]f32 = 4 banks; pp (shared small:
    # psT f8 [128,2048] and [8,512]f32 tiles) 2 banks; pR psc_rep 2 banks.
    pS = es.enter_context(tc.tile_pool(name="psS", bufs=2, space="PSUM"))
    pp = es.enter_context(tc.tile_pool(name="psT", bufs=2, space="PSUM"))
    pR = es.enter_context(tc.tile_pool(name="psR", bufs=1, space="PSUM"))

    # ---- DMA issue order ----
    # gpsimd (SWDGE, casts, 16-queue spread): h016, inp16, nat0, nat1, ...
    h016 = wp.tile([BPC, U], F16)
    nc.gpsimd.dma_start(out=h016, in_=h0_d)
    inp16 = wp.tile([BPC, EMB], F16)
    nc.gpsimd.dma_start(out=inp16, in_=inp_d)

    nat_pre = {}
    for pb_ in range(2):
        t = natp.tile([128, TC, U], F16, tag="nat", name=f"natp{pb_}")
        nc.gpsimd.dma_start(out=t, in_=bass.AP(
            ctx_d.tensor, pb_ * T * U, [[TC * U, 128], [U, TC], [1, U]]))
        nat_pre[pb_] = t

    # sync queue: small consts first, big rec/attk last.
    id16 = wp.tile([128, 128], F16)
    nc.sync.dma_start(out=id16, in_=id16_d)
    va8_sb = wp.tile([128, KU, 128], F8)
    nc.sync.dma_start(out=va8_sb, in_=va8_d)
    wa8_sb = wp.tile([128, 2, 2, KU, 128], F8)
    nc.sync.dma_start(out=wa8_sb, in_=wa8_d)
    bavr = wp.tile([128, 1], F32)
    nc.sync.dma_start(out=bavr, in_=bass.AP(bav_d.tensor, 0, [[0, 128], [1, 1]]))
    bawt8 = wp.tile([128, KU, BPC], F32)
    nc.sync.dma_start(out=bawt8, in_=bawt_d)
    bau8 = wp.tile([BPC, U], F32)
    nc.sync.dma_start(out=bau8, in_=_bcast_rows(bau_d, BPC, U))
    bi8 = wp.tile([BPC, U], F32)
    nc.sync.dma_start(out=bi8, in_=_bcast_rows(bi_d, BPC, U))
    bg8 = wp.tile([BPC, 3 * U], F32)
    nc.sync.dma_start(out=bg8, in_=_bcast_rows(bg_d, BPC, 3 * U))
    bo8 = wp.tile([BPC, U], F32)
    nc.sync.dma_start(out=bo8, in_=_bcast_rows(bo_d, BPC, U))
    h032 = wp.tile([BPC, U], F32)
    nc.sync.dma_start(out=h032, in_=h0_d)

    def load_kxm(dram, rows, cols, tag, engine):
        t = wp.tile([128, rows // 128, cols], F16, tag=tag, name=tag)
        src = bass.AP(dram.tensor, 0, [[cols, 128], [128 * cols, rows // 128], [1, cols]])
        engine.dma_start(out=t, in_=src)
        return t

    rec_sb = load_kxm(rec_d, U, 3 * U, "recw", nc.sync)
    attk_sb = load_kxm(attk_d, U, 3 * U, "attkw", nc.sync)

    # scalar(ACT) hwdge queue: ua/wi first (phase0 critical), then kern, wo.
    ua_sb = load_kxm(ua_d, U, U, "uaw", nc.scalar)
    wi_sb = load_kxm(wi_d, EMB, U, "wiw", nc.scalar)
    kern_sb = load_kxm(kern_d, U, 3 * U, "kernw", nc.scalar)
    wo_sb = load_kxm(wo_d, U, U, "wow", nc.scalar)

    # ---- persistent intermediates ----
    qb = wp.tile([128, KU, BPC], F32)       # tanh bias (q + ba_w)^T
    xg8 = wp.tile([BPC, 3 * U], F32)        # x@kernel + bias
    xgrz8 = wp.tile([BPC, 2 * U], F32)      # xg_z/r + h@R_z/r

    def transpose_to(dst, src, nrow, chunks, ident):
        """PE-transpose src [nrow, chunks*128] -> dst [128, chunks, nrow]."""
        pm = pp.tile([128, chunks * nrow], src.dtype, tag="u", name="pm")
        for c in range(chunks):
            nc.tensor.transpose(
                pm[:, c * nrow:(c + 1) * nrow],
                src[0:nrow, c * 128:(c + 1) * 128],
                ident[0:nrow, 0:nrow],
            )
        nc.vector.tensor_copy(dst, pm[:, 0:chunks * nrow])

    def mm8(lhsT, rhs_w, ncol_off, n=U):
        """[BPC, n] = lhsT^T @ rhs_w[:, :, off:off+n], accumulated over KU chunks."""
        ptile = pp.tile([BPC, n], F32, tag="u", name="p8")
        for c in range(KU):
            nc.tensor.matmul(ptile, lhsT[:, c, :],
                             rhs_w[:, c, ncol_off:ncol_off + n],
                             start=(c == 0), stop=(c == KU - 1))
        return ptile

    # ---- phase0-A: q = h @ Ua + ba_u -> qb (critical for first tanh) ----
    hT = wp.tile([128, KU, BPC], F16)
    transpose_to(hT, h016, BPC, KU, id16)
    pq = mm8(hT, ua_sb, 0)
    q16 = wp.tile([BPC, U], F16, tag="q16", name="q16")
    nc.vector.tensor_add(q16, pq, bau8)
    pmq = pp.tile([128, KU * BPC], F16, tag="u", name="pmq")
    for c in range(KU):
        nc.tensor.transpose(pmq[:, c * BPC:(c + 1) * BPC],
                            q16[0:BPC, c * 128:(c + 1) * 128],
                            id16[0:BPC, 0:BPC])
    for c in range(KU):
        nc.vector.tensor_add(qb[:, c, :], pmq[:, c * BPC:(c + 1) * BPC],
                             bawt8[:, c, :])

    # phase0-B part 1: x = inputs @ Wi + bi (xT needed for deferred xg)
    inT = wp.tile([128, 2, BPC], F16)
    transpose_to(inT, inp16, BPC, 2, id16)
    px = pp.tile([BPC, U], F32, tag="u", name="px")
    for c in range(2):
        nc.tensor.matmul(px, inT[:, c, :], wi_sb[:, c, :],
                         start=(c == 0), stop=(c == 1))
    x16 = wp.tile([BPC, U], F16, tag="x16", name="x16")
    nc.vector.tensor_add(x16, px, bi8)
    xT = wp.tile([128, KU, BPC], F16)
    transpose_to(xT, x16, BPC, KU, id16)

    def emit_phase0_rest():
        # xg = x @ kernel + bias ; xgrz = xg_zr + h @ R_zr  (M=8)
        for n in range(3):
            pg = mm8(xT, kern_sb, n * U)
            nc.vector.tensor_add(xg8[:, n * U:(n + 1) * U], pg,
                                 bg8[:, n * U:(n + 1) * U])
        for n in range(2):
            pr = mm8(hT, rec_sb, n * U)
            nc.vector.tensor_add(xgrz8[:, n * U:(n + 1) * U], pr,
                                 xg8[:, n * U:(n + 1) * U])

    # ---- streaming over batches ----
    # nat[p, j, u] = ctx[b, 16p+j, u] in f8: all t-indexing downstream
    # inherits this scrambled order consistently (softmax is permutation-
    # invariant), so correctness is unaffected.
    cvT16 = wp.tile([128, KU, BPC], F16, tag="cvT16", name="cvT16")

    phase0_done = False
    for b in range(BPC):
        if b in nat_pre:
            nat = nat_pre.pop(b)
        else:
            nat = natp.tile([128, TC, U], F16, tag="nat")
        if b + 2 < BPC:
            nat_pre[b + 2] = natp.tile([128, TC, U], F16, tag="nat",
                                       name=f"natp{b + 2}")
            nc.gpsimd.dma_start(out=nat_pre[b + 2], in_=bass.AP(
                ctx_d.tensor, (b + 2) * T * U, [[TC * U, 128], [U, TC], [1, U]]))

        zp = bp.tile([128, 2], F32, tag="zpb")
        cvPart = bp.tile([128, KU], F32, tag="cvPart")
        cvPartB = bp.tile([128, KU], F32, tag="cvPartB")

        for th in range(2):
            natT = ntp.tile([128, KU, TH], F8, tag="natT")

            # f16 transposes per uc; f16->f8 unpack copies split DVE/ACT
            for uc in range(KU):
                psT = pp.tile([128, 8, 128], F16, tag="u", name="psT")
                for j in range(8):
                    nc.tensor.transpose(
                        psT[:, j, :],
                        nat[:, th * 8 + j, uc * 128:(uc + 1) * 128],
                        id16,
                    )
                if uc < 2:
                    nc.vector.tensor_copy(natT[:, uc, :], psT)
                else:
                    nc.scalar.copy(natT[:, uc, :], psT)

            # scores: S^T chunks in PSUM via f8 DoubleRow, tanh -> th8 (f8)
            th8 = thp.tile([128, KU, TH], F8, tag="th8")

            def score_mms(ps_tiles, ms, half):
                for mi, m in enumerate(ms):
                    for c in range(2):
                        nc.tensor.matmul(
                            ps_tiles[mi][:, half * 512:(half + 1) * 512],
                            wa8_sb[:, c, :, m, :],
                            natT[:, 2 * c:2 * c + 2, half * 512:(half + 1) * 512],
                            start=(c == 0), stop=(c == 1),
                            perf_mode=DR,
                        )

            ps01 = [pS.tile([128, TH], F32, tag="S", name=f"ps{mm}") for mm in range(2)]
            score_mms(ps01, [0, 1], 0)
            score_mms(ps01, [0, 1], 1)
            for mi, m in enumerate([0, 1]):
                nc.scalar.activation(th8[:, m, :], ps01[mi], AF.Tanh,
                                     scale=1.0 / 16.0, bias=qb[:, m, b:b + 1])
            ps23 = [pS.tile([128, TH], F32, tag="S", name=f"ps{mm + 2}") for mm in range(2)]
            score_mms(ps23, [2, 3], 0)
            score_mms(ps23, [2, 3], 1)
            for mi, m in enumerate([2, 3]):
                nc.scalar.activation(th8[:, m, :], ps23[mi], AF.Tanh,
                                     scale=1.0 / 16.0, bias=qb[:, m, b:b + 1])

            # sc (replicated across partitions) = 16*(S@Va) via DoubleRow f8
            psc = pR.tile([128, TH], F32, tag="sc")
            for half in range(2):
                for c in range(2):
                    nc.tensor.matmul(psc[:, half * 512:(half + 1) * 512],
                                     va8_sb[:, 2 * c:2 * c + 2, :],
                                     th8[:, 2 * c:2 * c + 2, half * 512:(half + 1) * 512],
                                     start=(c == 0), stop=(c == 1), perf_mode=DR)

            # exp (+accumulate normalizer per partition)
            expRep = erp.tile([128, TH], F16, tag="expRep")
            nc.scalar.activation(expRep, psc, AF.Exp, scale=1.0 / 16.0,
                                 bias=bavr, accum_out=zp[:, th:th + 1])

            # cv partial on DVE: cv[u] += sum_t natT[u,t]*exp[t]
            dump = erp.tile([128, TH], F16, tag="dump")
            cvdst = cvPart if th == 0 else cvPartB
            for uc in range(KU):
                nc.vector.scalar_tensor_tensor(
                    dump, natT[:, uc, :], 1.0, expRep,
                    OP.mult, OP.mult, accum_out=cvdst[:, uc:uc + 1])

        # 1/Z and cv^T column for this batch
        zrec = bp.tile([128, 1], F32, tag="zrec")
        nc.vector.tensor_add(zrec, zp[:, 0:1], zp[:, 1:2])
        nc.vector.reciprocal(zrec, zrec)
        cvs = bp.tile([128, KU], F32, tag="cvs")
        nc.vector.tensor_add(cvs, cvPart, cvPartB)
        nc.vector.tensor_scalar(cvT16[:, :, b:b + 1], cvs, zrec, None, OP.mult)

        if not phase0_done and b >= 1:
            emit_phase0_rest()
            phase0_done = True

    # ---- post: gates, h, out for all 8 batches at once ----
    def sigmoid8(dst, pre):
        t1 = gp.tile([BPC, U], F32, tag="sig_t")
        nc.scalar.activation(t1, pre, AF.Tanh, scale=0.5)
        nc.vector.tensor_scalar(dst, t1, 0.5, 0.5, OP.mult, OP.add)

    # z gate
    pcg_z = mm8(cvT16, attk_sb, 0)
    zpre = gp.tile([BPC, U], F32, tag="zpre")
    nc.vector.scalar_tensor_tensor(zpre, pcg_z, 1.0, xgrz8[:, 0:U],
                                   OP.mult, OP.add)
    zg = gp.tile([BPC, U], F32, tag="zg")
    sigmoid8(zg, zpre)

    # r gate
    pcg_r = mm8(cvT16, attk_sb, U)
    rpre = gp.tile([BPC, U], F32, tag="rpre")
    nc.vector.scalar_tensor_tensor(rpre, pcg_r, 1.0, xgrz8[:, U:2 * U],
                                   OP.mult, OP.add)
    rg = gp.tile([BPC, U], F32, tag="rg")
    sigmoid8(rg, rpre)

    # rec_h = (r*h) @ Rh
    rh16 = gp.tile([BPC, U], F16, tag="rh16")
    nc.vector.tensor_mul(rh16, rg, h032)
    rhT = gp.tile([128, KU, BPC], F16, tag="rhT")
    transpose_to(rhT, rh16, BPC, KU, id16)
    prh = mm8(rhT, rec_sb, 2 * U)

    # h_bar
    hpre = gp.tile([BPC, U], F32, tag="hpre")
    nc.vector.scalar_tensor_tensor(hpre, prh, 1.0, xg8[:, 2 * U:3 * U],
                                   OP.mult, OP.add)
    pcg_h = mm8(cvT16, attk_sb, 2 * U)
    nc.vector.tensor_add(hpre, hpre, pcg_h)
    hbar = gp.tile([BPC, U], F32, tag="hbar")
    nc.scalar.activation(hbar, hpre, AF.Tanh)

    # h = hbar + z*(h_tm1 - hbar)
    dd = gp.tile([BPC, U], F32, tag="dd")
    nc.vector.tensor_sub(dd, h032, hbar)
    h_out = gp.tile([BPC, U], F32, tag="h_out")
    nc.vector.scalar_tensor_tensor(h_out, dd, 1.0, zg, OP.mult, OP.mult)
    nc.vector.tensor_add(h_out, h_out, hbar)
    nc.sync.dma_start(out=h_d, in_=h_out)

    # out = h @ Wo + bo
    h16 = gp.tile([BPC, U], F16, tag="h16")
    nc.vector.tensor_copy(h16, h_out)
    hT4 = gp.tile([128, KU, BPC], F16, tag="hT4")
    transpose_to(hT4, h16, BPC, KU, id16)
    pout = mm8(hT4, wo_sb, 0)
    o_out = gp.tile([BPC, U], F32, tag="o_out")
    nc.vector.tensor_add(o_out, pout, bo8)
    nc.sync.dma_start(out=out_d, in_=o_out)

    es.close()


_PROGRAM = None


def _get_program():
    global _PROGRAM
    if _PROGRAM is None:
        _PROGRAM = _build_program()
    return _PROGRAM


def make_in_maps(inputs, h_tm1, context, Wi, bi, kernel, recurrent_kernel,
                 attention_kernel, bias, Wa, ba_w, Ua, ba_u, Va, ba_v, Wo, bo):
    f32 = lambda x: np.ascontiguousarray(np.asarray(x, dtype=np.float32))
    f16 = lambda x: np.ascontiguousarray(np.asarray(x, dtype=np.float32).astype(np.float16))
    f8np = mybir.dt.np(F8)

    context = f32(context)
    inputs = f32(inputs)
    h_tm1 = f32(h_tm1)

    wa32 = np.asarray(Wa, np.float32) * 16.0
    wa8dr = np.zeros((128, 2, 2, KU, 128), np.float32)
    for c in range(2):
        for i in range(2):
            for mc in range(KU):
                # lhsT[p, i, m] = Wa'[c*256 + i*128 + p, mc*128 + m]
                wa8dr[:, c, i, mc, :] = wa32[c * 256 + i * 128: c * 256 + (i + 1) * 128,
                                             mc * 128:(mc + 1) * 128]
    # va8rep[p, m, j] = 16*Va[m*128+p] for all j (partition-replicated output)
    va16 = (np.asarray(Va, np.float32).reshape(KU, 128) * 16.0)
    va8rep = np.repeat(va16.transpose(1, 0)[:, :, None], 128, axis=2)

    shared = {
        "wa8dr": np.ascontiguousarray(wa8dr.astype(f8np)),
        "va8rep": np.ascontiguousarray(va8rep.astype(f8np)),
        "ident16": np.eye(128, dtype=np.float16),
        "ua16": f16(Ua), "wi16": f16(Wi),
        "kern16": f16(kernel), "rec16": f16(recurrent_kernel),
        "attk16": f16(attention_kernel), "wo16": f16(Wo),
        "bi": f32(bi), "biasg": f32(bias), "ba_u": f32(ba_u),
        "ba_wt8": np.ascontiguousarray(np.repeat(
            np.asarray(ba_w, np.float32).reshape(KU, 128).T[:, :, None], BPC, axis=2)),
        "ba_v1": f32(ba_v).reshape(1, 1),
        "bo": f32(bo),
    }
    in_maps = []
    for i in range(NCORES):
        s = slice(i * BPC, (i + 1) * BPC)
        in_maps.append({
            "ctx": context[s], "inp": inputs[s], "h0": h_tm1[s], **shared,
        })
    return in_maps


def kernel(**inputs):
    from concourse.bass_utils import run_bass_kernel_spmd

    nc = _get_program()
    in_maps = make_in_maps(**inputs)
    res = run_bass_kernel_spmd(nc, in_maps, list(range(NCORES)))
    out = np.concatenate([r["out_o"] for r in res.results], axis=0)
    h = np.concatenate([r["h_o"] for r in res.results], axis=0)
    return out.astype(np.float32), h.astype(np.float32)


if __name__ == "__main__":
    prog = _get_program()
    print("program built OK")
